# revision 6
# baseline (speedup 1.0000x reference)
"""Poincare pairwise edge generator on 8 Trainium2 NeuronCores (v3).

Math (c=1): S = s + u - 2<x,y>, D = 1 - 2<x,y> + s*u  (s=|x|^2, u=|y|^2)
  z = sqrt(S/D);  dists = ln(1+z) - ln(1-z);  probs = (1-z)/2 = exp(ln(1-z))/2
computed entirely with Ln/Exp (one ACT table set -> zero table reloads):
  a = Ln(X*2^-10)        X = 2^10*S   (in-place)
  b = Ln(Y*2^-10 + 1)    Y = 2^10*(s*u - 2<x,y>)   (in-place)
  H = a - b  [DVE]; z = Exp(0.5*H); LP = Ln(z+1); LM = Ln(1-z)
  probs = Exp(LM - ln2)  [bf16]; dists = LP - LM  [DVE, bf16]

The matmul computes ps = 2^10*(-2<x_i,x_j>) with ONE fp8e4m3 DoubleRow
matmul per 512 columns (K=256 in a single instruction, both sides
prescaled by 32).  Extraction adds the rank-1 terms straight off PSUM:
  sttA: X = (ubc + 2^10*s_i) + ps      sttB: Y = (ubc * s_i) + ps
with ubc = 2^10*u_j broadcast (bf16).  This environment charges
~25-130us PER INSTRUCTION regardless of operand width and engines
serialize, so the design minimizes total instruction count.

Symmetry: only the upper triangle is computed; host mirrors.  Core c
owns global row-blocks {8t+c}; block 8t+c covers columns [1024t, 8192)
-- identical program on every core (widths 8192..1024), only data
differs.  Since the post-extraction chain has NO per-partition scalars,
tiers are processed in 3 groups of exactly 12288 columns ({0,4}, {1,3},
{2,5,6,7}), so each chain op covers 12288 elements in one instruction.
Diagonal entries hit Ln(negative) (fp8 noise around S_ii=0) -> NaN;
the host triu mirror discards them and zeroes the diagonal exactly.
"""

import sys

sys.path.insert(0, '/opt/trn_rl_repo')

import numpy as np

_compiled = None

N_TOTAL = 8192
TIERS = 8
SC = 32.0
SC2 = SC * SC          # 1024 = 2^10
LN2 = 0.6931471805599453

TIER_W = [N_TOTAL - 1024 * t for t in range(TIERS)]
TIER_C0 = [1024 * t for t in range(TIERS)]
GROUPS = [[0, 4], [1, 3], [2, 5, 6, 7]]       # each sums to 12288 cols
GW = 12288


def _group_layout():
    """Per group: [(tier, loff, W, c0)]; psum tiles
    [(gi, tier, xoff_in_group, abs_col, w)]."""
    lay = []
    tiles = []
    for gi, g in enumerate(GROUPS):
        loff = 0
        lg = []
        for t in g:
            W = TIER_W[t]
            c0 = TIER_C0[t]
            lg.append((t, loff, W, c0))
            xo = 0
            while xo < W:
                w = min(4096, W - xo)
                tiles.append((gi, t, loff + xo, c0 + xo, w))
                xo += w
            loff += W
        assert loff == GW
        lay.append(lg)
    return lay, tiles


def _build_raw(reps=1, bench=False, tiny_io=False):
    import concourse.bass as bass
    import concourse.mybir as mybir

    DT = mybir.dt.float32
    BF = mybir.dt.bfloat16
    F8 = mybir.dt.float8e4
    F = mybir.ActivationFunctionType
    OP = mybir.AluOpType
    DR = mybir.MatmulPerfMode.DoubleRow

    nc = bass.Bass()

    NG = len(GROUPS)
    if tiny_io:
        nc.declare_dram_parameter("tiny", [128, 4], DT, isOutput=False)
        lhsT = nc.dram_tensor("lhsT", [128, 2, 1024], F8)
        rhs = nc.dram_tensor("rhs", [128, 2, N_TOTAL], F8)
        ubc = nc.dram_tensor("ubc", [128, N_TOTAL], BF)
        sv = nc.dram_tensor("sv", [128, TIERS, 4], DT)
        d_o = [nc.dram_tensor(f"d{g}", [128, GW], BF) for g in range(NG)]
        p_o = [nc.dram_tensor(f"p{g}", [128, GW], BF) for g in range(NG)]
        done_o = nc.declare_dram_parameter("done_o", [128, 4], DT,
                                           isOutput=True)
    else:
        lhsT = nc.declare_dram_parameter("lhsT", [128, 2, 1024], F8,
                                         isOutput=False)
        rhs = nc.declare_dram_parameter("rhs", [128, 2, N_TOTAL], F8,
                                        isOutput=False)
        ubc = nc.declare_dram_parameter("ubc", [128, N_TOTAL], BF,
                                        isOutput=False)
        sv = nc.declare_dram_parameter("sv", [128, TIERS, 4], DT,
                                       isOutput=False)
        d_o = [nc.declare_dram_parameter(f"d{g}", [128, GW], BF,
                                         isOutput=True) for g in range(NG)]
        p_o = [nc.declare_dram_parameter(f"p{g}", [128, GW], BF,
                                         isOutput=True) for g in range(NG)]
        done_o = None

    NIN = 4 * 16
    lay, tiles = _group_layout()
    NTILE = len(tiles)             # 11
    # cumulative tile count through group gi
    gtiles = [sum(1 for tl in tiles if tl[0] <= gi) for gi in range(NG)]
    gstart = [0] + gtiles[:-1]

    from contextlib import ExitStack
    with ExitStack() as ctx:
        block = ctx.enter_context(nc.Block())
        dma_in = ctx.enter_context(nc.semaphore("dma_in"))
        pe_s = ctx.enter_context(nc.semaphore("pe_s"))
        x_s = ctx.enter_context(nc.semaphore("x_s"))
        ab_s = ctx.enter_context(nc.semaphore("ab_s"))
        h_s = ctx.enter_context(nc.semaphore("h_s"))
        lm_s = ctx.enter_context(nc.semaphore("lm_s"))
        d_s = ctx.enter_context(nc.semaphore("d_s"))
        o_s = ctx.enter_context(nc.semaphore("o_s"))
        dma_o = ctx.enter_context(nc.semaphore("dma_o"))
        t_l = ctx.enter_context(nc.sbuf_tensor("t_l", [128, 2, 1024], F8))
        t_r = ctx.enter_context(nc.sbuf_tensor("t_r", [128, 2, N_TOTAL], F8))
        t_u = ctx.enter_context(nc.sbuf_tensor("t_u", [128, N_TOTAL], BF))
        t_sv = ctx.enter_context(nc.sbuf_tensor("t_sv", [128, TIERS, 4], DT))
        X = ctx.enter_context(nc.sbuf_tensor("X", [128, GW], DT))
        Y = ctx.enter_context(nc.sbuf_tensor("Y", [128, GW], DT))
        LB = ctx.enter_context(nc.sbuf_tensor("LB", [128, GW], BF))
        PB = ctx.enter_context(nc.sbuf_tensor("PB", [128, GW], BF))
        ps = ctx.enter_context(nc.psum_tensor("ps", [128, 4096], DT))

        @block.sync
        def _(sync):
            for t, src in [(t_l, lhsT), (t_r, rhs), (t_u, ubc), (t_sv, sv)]:
                sync.dma_start(out=t[:], in_=src[:]).then_inc(dma_in, 16)
            for r in range(reps):
                for gi in range(NG):
                    h = r * NG + gi
                    sync.wait_ge(d_s, h + 1)
                    sync.dma_start(out=d_o[gi][:],
                                   in_=LB[:]).then_inc(dma_o, 16)
                    sync.wait_ge(o_s, h + 1)
                    sync.dma_start(out=p_o[gi][:],
                                   in_=PB[:]).then_inc(dma_o, 16)
            sync.wait_ge(dma_o, 32 * NG * reps + (16 if bench else 0))

        @block.tensor
        def _(te):
            te.wait_ge(dma_in, NIN)
            for r in range(reps):
                for k, (gi, t, xoff, acol, w) in enumerate(tiles):
                    g = r * NTILE + k
                    if g >= 1:
                        te.wait_ge(x_s, g)
                    lsl = t_l[:, :, t * 128:(t + 1) * 128]
                    for s in range(0, w, 512):
                        mm = te.matmul(ps[:, s:s + 512], lsl,
                                       t_r[:, :, acol + s:acol + s + 512],
                                       start=True, stop=True, perf_mode=DR)
                    mm.then_inc(pe_s, 1)

        @block.vector
        def _(v):
            for r in range(reps):
                for gi in range(NG):
                    h = r * NG + gi
                    if h >= 1:
                        v.wait_ge(o_s, h)      # X/Y free (prev grp probs done)
                    for kk in range(gstart[gi], gtiles[gi]):
                        _, t, xoff, acol, w = tiles[kk]
                        g = r * NTILE + kk
                        v.wait_ge(pe_s, g + 1)
                        v.scalar_tensor_tensor(
                            out=X[:, xoff:xoff + w],
                            in0=t_u[:, acol:acol + w],
                            scalar=t_sv[:, t, 0:1],
                            in1=ps[:, 0:w], op0=OP.add, op1=OP.add)
                        v.scalar_tensor_tensor(
                            out=Y[:, xoff:xoff + w],
                            in0=t_u[:, acol:acol + w],
                            scalar=t_sv[:, t, 1:2],
                            in1=ps[:, 0:w], op0=OP.mult,
                            op1=OP.add).then_inc(x_s, 1)
                    # H = a - b   (ACT wrote a into X, b into Y)
                    v.wait_ge(ab_s, h + 1)
                    v.tensor_sub(out=X[:], in0=X[:],
                                 in1=Y[:]).then_inc(h_s, 1)
                    # dists = LP - LM   (bf16)
                    v.wait_ge(lm_s, h + 1)
                    if h >= 1:
                        v.wait_ge(dma_o, 32 * h - 16)   # LB DMA of prev grp
                    v.tensor_sub(out=LB[:], in0=X[:],
                                 in1=Y[:]).then_inc(d_s, 1)

        @block.scalar
        def _(sc):
            for r in range(reps):
                for gi in range(NG):
                    h = r * NG + gi
                    sc.wait_ge(x_s, r * NTILE + gtiles[gi])
                    sc.activation(X[:], X[:], F.Ln, bias=0.0, scale=1.0 / SC2)
                    sc.activation(Y[:], Y[:], F.Ln, bias=1.0,
                                  scale=1.0 / SC2).then_inc(ab_s, 1)
                    sc.wait_ge(h_s, h + 1)
                    sc.activation(Y[:], X[:], F.Exp, bias=0.0, scale=0.5)
                    sc.activation(X[:], Y[:], F.Ln, bias=1.0, scale=1.0)
                    sc.activation(Y[:], Y[:], F.Ln, bias=1.0,
                                  scale=-1.0).then_inc(lm_s, 1)
                    if h >= 1:
                        sc.wait_ge(dma_o, 32 * h)       # PB DMA of prev grp
                    sc.activation(PB[:], Y[:], F.Exp,
                                  bias=t_sv[:, 0, 2:3],
                                  scale=1.0).then_inc(o_s, 1)

        @block.gpsimd
        def _(gp):
            if bench:
                gp.wait_ge(o_s, NG * reps)
                gp.memset(X[:, 0:4], 0.0)
                gp.dma_start(out=done_o[:],
                             in_=X[:, 0:4]).then_inc(dma_o, 16)

    return nc


def _prepare_in_maps(embeddings):
    import ml_dtypes
    f8 = ml_dtypes.float8_e4m3
    bf = ml_dtypes.bfloat16

    E = np.ascontiguousarray(embeddings, dtype=np.float32)
    u = (E.astype(np.float64) ** 2).sum(axis=1)          # [n]

    ET = E.T.astype(np.float64)                          # [256, n]
    rhs = np.ascontiguousarray(
        (SC * ET).astype(np.float32).astype(f8)
        .reshape(2, 128, N_TOTAL).transpose(1, 0, 2))
    lhs_all = (-2.0 * SC * ET).astype(np.float32).astype(f8)   # [256, n]

    ubc = np.ascontiguousarray(np.broadcast_to(
        (SC2 * u).astype(np.float32).astype(bf)[None, :], (128, N_TOTAL)))

    in_maps = []
    for c in range(8):
        gidx = np.concatenate(
            [np.arange((8 * t + c) * 128, (8 * t + c) * 128 + 128)
             for t in range(TIERS)])
        lhsT = np.ascontiguousarray(
            lhs_all[:, gidx].reshape(2, 128, 1024).transpose(1, 0, 2))
        sv = np.zeros((128, TIERS, 4), np.float32)
        for t in range(TIERS):
            sblk = u[(8 * t + c) * 128:(8 * t + c) * 128 + 128]
            sv[:, t, 0] = SC2 * sblk          # sttA: ubc + 2^10*s
            sv[:, t, 1] = sblk                # sttB: ubc * s
        sv[:, :, 2] = -LN2                    # probs Exp bias
        in_maps.append({"lhsT": lhsT, "rhs": rhs, "ubc": ubc,
                        "sv": np.ascontiguousarray(sv)})
    return in_maps


def kernel(embeddings: np.ndarray) -> tuple[np.ndarray, np.ndarray]:
    global _compiled
    from concourse.bass_utils import run_bass_kernel_spmd

    if _compiled is None:
        _compiled = _build_raw()
    nc = _compiled

    in_maps = _prepare_in_maps(embeddings)
    res = run_bass_kernel_spmd(nc, in_maps, list(range(8)))

    lay, _ = _group_layout()
    dists = np.zeros((N_TOTAL, N_TOTAL), np.float32)
    probs = np.zeros((N_TOTAL, N_TOTAL), np.float32)
    for c in range(8):
        for gi, lg in enumerate(lay):
            dg = np.asarray(res.results[c][f"d{gi}"]).astype(np.float32)
            pg = np.asarray(res.results[c][f"p{gi}"]).astype(np.float32)
            for (t, loff, W, c0) in lg:
                r0 = (8 * t + c) * 128
                dists[r0:r0 + 128, c0:c0 + W] = dg[:, loff:loff + W]
                probs[r0:r0 + 128, c0:c0 + W] = pg[:, loff:loff + W]

    du = np.triu(dists, 1)
    dists = du + du.T
    pu = np.triu(probs, 1)
    probs = pu + pu.T
    return (probs, dists)


# revision 8
# speedup vs baseline: 1.2721x; 1.2721x over previous
"""Poincare pairwise edge generator on 8 Trainium2 NeuronCores (v4).

Math (c=1): S = s + u - 2<x,y>, D = 1 - 2<x,y> + s*u  (s=|x|^2, u=|y|^2)
  z = sqrt(S/D);  dists = ln(1+z) - ln(1-z);  probs = (1-z)/2
computed with Ln/Exp only (both live in ONE ACT table set, and
Identity/Copy live in every set, so table reloads are rare):
  ext:  X = Identity(ps + 2^10*s_i)   [ACT reads PSUM -- cheap; DVE
        reads of fresh PSUM carry a big per-instruction premium here]
  u:    X += 2^10*u_j                 [DVE, per tier]
  sttB: Y = (ubc * -(1-s_i)) + X      [DVE, per tier, SBUF only]
  b:    Y = Ln(Y*2^-10 + (1-s_i))     [= ln D, per tier (AP bias)]
  a:    X = Ln(X*2^-10)               [= ln S, per group]
  H = a - b [DVE]; z = Exp(0.5H) -> PB bf16; LP = Ln(PB+1) -> X;
  LM = Ln(1-PB) -> Y; dists = X - Y -> LB bf16 [DVE];
  probs = PB*-0.5 + 0.5 in-place [DVE ts]

The matmul computes ps = 2^10*(-2<x_i,x_j>) with ONE fp8e4m3 DoubleRow
matmul per 512 columns (K=256 per instruction, both sides prescaled by
32).  This environment charges ~25-130us PER INSTRUCTION regardless of
operand width and engines serialize, so the design minimizes total
instruction count and keeps DMA waits off the critical path by
double-buffering the bf16 output staging buffers.

Symmetry: only the upper triangle is computed; host mirrors.  Core c
owns global row-blocks {8t+c}; block 8t+c covers columns [1024t, 8192)
-- identical program on every core, only data differs.  Tiers are
processed in 4 groups of exactly 9216 columns ({0,7},{1,6},{2,5},{3,4})
so group-level chain ops cover 9216 elements per instruction.
Diagonal entries hit Ln(negative) (fp8 noise around S_ii=0) -> NaN;
the host triu mirror discards them and zeroes the diagonal exactly.
"""

import sys

sys.path.insert(0, '/opt/trn_rl_repo')

import numpy as np

_compiled = None

N_TOTAL = 8192
TIERS = 8
SC = 32.0
SC2 = SC * SC          # 1024 = 2^10

TIER_W = [N_TOTAL - 1024 * t for t in range(TIERS)]
TIER_C0 = [1024 * t for t in range(TIERS)]
GROUPS = [[0, 7], [1, 6], [2, 5], [3, 4]]     # each sums to 9216 cols
GW = 9216


def _group_layout():
    """Per group: [(tier, loff, W, c0)]; psum tiles
    [(gi, tier, xoff_in_group, abs_col, w)]."""
    lay = []
    tiles = []
    for gi, g in enumerate(GROUPS):
        loff = 0
        lg = []
        for t in g:
            W = TIER_W[t]
            c0 = TIER_C0[t]
            lg.append((t, loff, W, c0))
            xo = 0
            while xo < W:
                w = min(4096, W - xo)
                tiles.append((gi, t, loff + xo, c0 + xo, w))
                xo += w
            loff += W
        assert loff == GW
        lay.append(lg)
    return lay, tiles


def _build_raw(reps=1, bench=False, tiny_io=False):
    import concourse.bass as bass
    import concourse.mybir as mybir

    DT = mybir.dt.float32
    BF = mybir.dt.bfloat16
    F8 = mybir.dt.float8e4
    F = mybir.ActivationFunctionType
    OP = mybir.AluOpType
    DR = mybir.MatmulPerfMode.DoubleRow

    nc = bass.Bass()

    NG = len(GROUPS)
    if tiny_io:
        nc.declare_dram_parameter("tiny", [128, 4], DT, isOutput=False)
        lhsT = nc.dram_tensor("lhsT", [128, 2, 1024], F8)
        rhs = nc.dram_tensor("rhs", [128, 2, N_TOTAL], F8)
        ubc = nc.dram_tensor("ubc", [128, N_TOTAL], BF)
        sv = nc.dram_tensor("sv", [128, TIERS, 4], DT)
        d_o = [nc.dram_tensor(f"d{g}", [128, GW], BF) for g in range(NG)]
        p_o = [nc.dram_tensor(f"p{g}", [128, GW], BF) for g in range(NG)]
        done_o = nc.declare_dram_parameter("done_o", [128, 4], DT,
                                           isOutput=True)
    else:
        lhsT = nc.declare_dram_parameter("lhsT", [128, 2, 1024], F8,
                                         isOutput=False)
        rhs = nc.declare_dram_parameter("rhs", [128, 2, N_TOTAL], F8,
                                        isOutput=False)
        ubc = nc.declare_dram_parameter("ubc", [128, N_TOTAL], BF,
                                        isOutput=False)
        sv = nc.declare_dram_parameter("sv", [128, TIERS, 4], DT,
                                       isOutput=False)
        d_o = [nc.declare_dram_parameter(f"d{g}", [128, GW], BF,
                                         isOutput=True) for g in range(NG)]
        p_o = [nc.declare_dram_parameter(f"p{g}", [128, GW], BF,
                                         isOutput=True) for g in range(NG)]
        done_o = None

    NIN = 4 * 16
    lay, tiles = _group_layout()
    NTILE = len(tiles)             # 12
    # cumulative tile index through (group, tier)
    cum_tier = {}                  # (gi, t) -> cumulative tile count
    cnt = 0
    for (gi, t, xoff, acol, w) in tiles:
        cnt += 1
        cum_tier[(gi, t)] = cnt

    from contextlib import ExitStack
    with ExitStack() as ctx:
        block = ctx.enter_context(nc.Block())
        dma_in = ctx.enter_context(nc.semaphore("dma_in"))
        pe_s = ctx.enter_context(nc.semaphore("pe_s"))
        c_s = ctx.enter_context(nc.semaphore("c_s"))
        ey_s = ctx.enter_context(nc.semaphore("ey_s"))
        ab_s = ctx.enter_context(nc.semaphore("ab_s"))
        h_s = ctx.enter_context(nc.semaphore("h_s"))
        lm_s = ctx.enter_context(nc.semaphore("lm_s"))
        d_s = ctx.enter_context(nc.semaphore("d_s"))
        dma_o = ctx.enter_context(nc.semaphore("dma_o"))
        t_l = ctx.enter_context(nc.sbuf_tensor("t_l", [128, 2, 1024], F8))
        t_r = ctx.enter_context(nc.sbuf_tensor("t_r", [128, 2, N_TOTAL], F8))
        t_u = ctx.enter_context(nc.sbuf_tensor("t_u", [128, N_TOTAL], BF))
        t_sv = ctx.enter_context(nc.sbuf_tensor("t_sv", [128, TIERS, 4], DT))
        X = ctx.enter_context(nc.sbuf_tensor("X", [128, GW], DT))
        Y = ctx.enter_context(nc.sbuf_tensor("Y", [128, GW], DT))
        LB0 = ctx.enter_context(nc.sbuf_tensor("LB0", [128, GW], BF))
        LB1 = ctx.enter_context(nc.sbuf_tensor("LB1", [128, GW], BF))
        PB0 = ctx.enter_context(nc.sbuf_tensor("PB0", [128, GW], BF))
        PB1 = ctx.enter_context(nc.sbuf_tensor("PB1", [128, GW], BF))
        ps = ctx.enter_context(nc.psum_tensor("ps", [128, 4096], DT))

        LB = [LB0, LB1]
        PB = [PB0, PB1]

        @block.sync
        def _(sync):
            for t, src in [(t_l, lhsT), (t_r, rhs), (t_u, ubc), (t_sv, sv)]:
                sync.dma_start(out=t[:], in_=src[:]).then_inc(dma_in, 16)
            for r in range(reps):
                for gi in range(NG):
                    h = r * NG + gi
                    sync.wait_ge(d_s, 2 * h + 1)
                    sync.dma_start(out=d_o[gi][:],
                                   in_=LB[h % 2][:]).then_inc(dma_o, 16)
                    sync.wait_ge(d_s, 2 * h + 2)
                    sync.dma_start(out=p_o[gi][:],
                                   in_=PB[h % 2][:]).then_inc(dma_o, 16)
            sync.wait_ge(dma_o, 32 * NG * reps + (16 if bench else 0))

        @block.tensor
        def _(te):
            te.wait_ge(dma_in, NIN)
            for r in range(reps):
                for k, (gi, t, xoff, acol, w) in enumerate(tiles):
                    g = r * NTILE + k
                    if g >= 1:
                        te.wait_ge(c_s, g)
                    lsl = t_l[:, :, t * 128:(t + 1) * 128]
                    for s in range(0, w, 512):
                        mm = te.matmul(ps[:, s:s + 512], lsl,
                                       t_r[:, :, acol + s:acol + s + 512],
                                       start=True, stop=True, perf_mode=DR)
                    mm.then_inc(pe_s, 1)

        @block.scalar
        def _(sc):
            for r in range(reps):
                k = 0
                for gi in range(NG):
                    h = r * NG + gi
                    if h >= 1:
                        sc.wait_ge(d_s, 2 * h)   # X/Y/prev bufs consumed
                    # extract psum tiles: X = ps + 2^10*s_i
                    for kk in range(k, k + 3):
                        _, t, xoff, acol, w = tiles[kk]
                        g = r * NTILE + kk
                        sc.wait_ge(pe_s, g + 1)
                        sc.activation(X[:, xoff:xoff + w], ps[:, 0:w],
                                      F.Identity,
                                      bias=t_sv[:, t, 0:1],
                                      scale=1.0).then_inc(c_s, 1)
                    k += 3
                    # b per tier (after DVE u-add + sttB of that tier)
                    for j, (t, loff, W, c0) in enumerate(lay[gi]):
                        sc.wait_ge(ey_s, 2 * h + j + 1)
                        sc.activation(Y[:, loff:loff + W],
                                      Y[:, loff:loff + W], F.Ln,
                                      bias=t_sv[:, t, 2:3], scale=1.0 / SC2)
                    # a group-wide (DVE finished reading X for sttB)
                    sc.activation(X[:], X[:], F.Ln, bias=0.0,
                                  scale=1.0 / SC2).then_inc(ab_s, 1)
                    sc.wait_ge(h_s, h + 1)
                    if h >= 2:
                        sc.wait_ge(dma_o, 32 * (h - 1))   # PB[h%2] DMA done
                    sc.activation(PB[h % 2][:], X[:], F.Exp,
                                  bias=0.0, scale=0.5)
                    sc.activation(X[:], PB[h % 2][:], F.Ln,
                                  bias=1.0, scale=1.0)
                    sc.activation(Y[:], PB[h % 2][:], F.Ln,
                                  bias=1.0, scale=-1.0).then_inc(lm_s, 1)

        @block.vector
        def _(v):
            for r in range(reps):
                for gi in range(NG):
                    h = r * NG + gi
                    # per tier: X += 2^10*u ; Y = (ubc * -(1-s)) + X
                    for j, (t, loff, W, c0) in enumerate(lay[gi]):
                        v.wait_ge(c_s, r * NTILE + cum_tier[(gi, t)])
                        v.tensor_add(out=X[:, loff:loff + W],
                                     in0=X[:, loff:loff + W],
                                     in1=t_u[:, c0:c0 + W])
                        v.scalar_tensor_tensor(
                            out=Y[:, loff:loff + W],
                            in0=t_u[:, c0:c0 + W],
                            scalar=t_sv[:, t, 1:2],
                            in1=X[:, loff:loff + W],
                            op0=OP.mult, op1=OP.add).then_inc(ey_s, 1)
                    # H = a - b
                    v.wait_ge(ab_s, h + 1)
                    v.tensor_sub(out=X[:], in0=X[:],
                                 in1=Y[:]).then_inc(h_s, 1)
                    # dists = LP - LM ; probs = PB*-0.5 + 0.5
                    v.wait_ge(lm_s, h + 1)
                    if h >= 2:
                        v.wait_ge(dma_o, 32 * (h - 2) + 16)  # LB[h%2] DMA'd
                    v.tensor_sub(out=LB[h % 2][:], in0=X[:],
                                 in1=Y[:]).then_inc(d_s, 1)
                    v.tensor_scalar(out=PB[h % 2][:], in0=PB[h % 2][:],
                                    scalar1=-0.5, scalar2=0.5,
                                    op0=OP.mult,
                                    op1=OP.add).then_inc(d_s, 1)

        @block.gpsimd
        def _(gp):
            if bench:
                gp.wait_ge(d_s, 2 * NG * reps)
                gp.memset(X[:, 0:4], 0.0)
                gp.dma_start(out=done_o[:],
                             in_=X[:, 0:4]).then_inc(dma_o, 16)

    return nc


def _prepare_in_maps(embeddings):
    import ml_dtypes
    f8 = ml_dtypes.float8_e4m3
    bf = ml_dtypes.bfloat16

    E = np.ascontiguousarray(embeddings, dtype=np.float32)
    u = (E.astype(np.float64) ** 2).sum(axis=1)          # [n]

    ET = E.T.astype(np.float64)                          # [256, n]
    rhs = np.ascontiguousarray(
        (SC * ET).astype(np.float32).astype(f8)
        .reshape(2, 128, N_TOTAL).transpose(1, 0, 2))
    lhs_all = (-2.0 * SC * ET).astype(np.float32).astype(f8)   # [256, n]

    ubc = np.ascontiguousarray(np.broadcast_to(
        (SC2 * u).astype(np.float32).astype(bf)[None, :], (128, N_TOTAL)))

    in_maps = []
    for c in range(8):
        gidx = np.concatenate(
            [np.arange((8 * t + c) * 128, (8 * t + c) * 128 + 128)
             for t in range(TIERS)])
        lhsT = np.ascontiguousarray(
            lhs_all[:, gidx].reshape(2, 128, 1024).transpose(1, 0, 2))
        sv = np.zeros((128, TIERS, 4), np.float32)
        for t in range(TIERS):
            sblk = u[(8 * t + c) * 128:(8 * t + c) * 128 + 128]
            sv[:, t, 0] = SC2 * sblk          # ext bias: ps + 2^10*s
            sv[:, t, 1] = -(1.0 - sblk)       # sttB: ubc * -(1-s)
            sv[:, t, 2] = 1.0 - sblk          # b bias: +(1-s)
        in_maps.append({"lhsT": lhsT, "rhs": rhs, "ubc": ubc,
                        "sv": np.ascontiguousarray(sv)})
    return in_maps


def kernel(embeddings: np.ndarray) -> tuple[np.ndarray, np.ndarray]:
    global _compiled
    from concourse.bass_utils import run_bass_kernel_spmd

    if _compiled is None:
        _compiled = _build_raw()
    nc = _compiled

    in_maps = _prepare_in_maps(embeddings)
    res = run_bass_kernel_spmd(nc, in_maps, list(range(8)))

    lay, _ = _group_layout()
    dists = np.zeros((N_TOTAL, N_TOTAL), np.float32)
    probs = np.zeros((N_TOTAL, N_TOTAL), np.float32)
    for c in range(8):
        for gi, lg in enumerate(lay):
            dg = np.asarray(res.results[c][f"d{gi}"]).astype(np.float32)
            pg = np.asarray(res.results[c][f"p{gi}"]).astype(np.float32)
            for (t, loff, W, c0) in lg:
                r0 = (8 * t + c) * 128
                dists[r0:r0 + 128, c0:c0 + W] = dg[:, loff:loff + W]
                probs[r0:r0 + 128, c0:c0 + W] = pg[:, loff:loff + W]

    du = np.triu(dists, 1)
    dists = du + du.T
    pu = np.triu(probs, 1)
    probs = pu + pu.T
    return (probs, dists)


# revision 10
# speedup vs baseline: 1.8323x; 1.4404x over previous
"""Poincare pairwise edge generator on 8 Trainium2 NeuronCores (v5).

Math (c=1): S = s + u - 2<x,y>, D = 1 - 2<x,y> + s*u  (s=|x|^2, u=|y|^2)
  z = sqrt(S/D);  dists = ln(1+z) - ln(1-z);  probs = (1-z)/2
computed with Ln/Exp only (one ACT table set -> no table reloads):
  sttA: X = ubc + ps                  [DVE; X = 2^10*(u - 2<x,y>)]
  sttB: Y = (ubc * -(1-s_i)) + X      [DVE per tier; Y = 2^10*(su-2dot)]
  a:    X = Ln(X*2^-10 + s_i)         [= ln S, per tier, AP bias]
  b:    Y = Ln(Y*2^-10 + 1)           [= ln D, per group]
  H = a - b [DVE]; z = Exp(0.5H) -> PB bf16; LP = Ln(PB+1) -> X;
  LM = Ln(1-PB) -> Y; dists = X - Y -> LB bf16 [DVE];
  probs = PB*-0.5 + 0.5 in-place [DVE ts]

The matmul computes ps = 2^10*(-2<x_i,x_j>) with ONE fp8e4m3 DoubleRow
matmul per 512 columns (K=256 per instruction, both sides prescaled by
32).  Cost model measured for this environment: ~30-80us dispatch per
instruction regardless of width, ~300us premium per PSUM-reading
instruction, ~20ns per element-column, engines serialize.  Hence: the
minimum 9 PSUM-extraction reads (9 tiles x 4096 = 36864 columns), the
fewest possible element passes, and group-wide ops at 12288 columns.

Symmetry: only the upper triangle is computed; host mirrors.  Core c
owns global row-blocks {8t+c}; block 8t+c covers columns [1024t, 8192)
-- identical program on every core, only data differs.  Tiers are
processed in 3 groups of exactly 12288 columns ({0,4},{1,3},{2,5,6,7}).
Output staging buffers are double-buffered so DMA completion waits
carry two groups of slack.  Diagonal entries hit Ln(negative) (fp8
noise around S_ii=0) -> NaN; the host triu mirror discards them and
zeroes the diagonal exactly.
"""

import sys

sys.path.insert(0, '/opt/trn_rl_repo')

import numpy as np

_compiled = None

N_TOTAL = 8192
TIERS = 8
SC = 32.0
SC2 = SC * SC          # 1024 = 2^10

TIER_W = [N_TOTAL - 1024 * t for t in range(TIERS)]
TIER_C0 = [1024 * t for t in range(TIERS)]
GROUPS = [[0, 4], [1, 3], [2, 5, 6, 7]]       # each sums to 12288 cols
GW = 12288


def _group_layout():
    """lay: per group [(tier, loff, W, c0)].
    tiles: flat 4096-wide psum tiles [(gi, xoff_in_group)].
    spans: per (gi, tier): [(xoff, acol, w)] pieces for column-mapped ops."""
    lay = []
    tiles = []
    for gi, g in enumerate(GROUPS):
        loff = 0
        lg = []
        for t in g:
            lg.append((t, loff, TIER_W[t], TIER_C0[t]))
            loff += TIER_W[t]
        assert loff == GW
        lay.append(lg)
        for xo in range(0, GW, 4096):
            tiles.append((gi, xo))
    return lay, tiles


def _col_of(lay, gi, xoff):
    """Map group-local offset -> (tier, absolute column)."""
    for (t, loff, W, c0) in lay[gi]:
        if loff <= xoff < loff + W:
            return t, c0 + (xoff - loff)
    raise ValueError


def _build_raw(reps=1, bench=False, tiny_io=False):
    import concourse.bass as bass
    import concourse.mybir as mybir

    DT = mybir.dt.float32
    BF = mybir.dt.bfloat16
    F8 = mybir.dt.float8e4
    F = mybir.ActivationFunctionType
    OP = mybir.AluOpType
    DR = mybir.MatmulPerfMode.DoubleRow

    nc = bass.Bass()

    NG = len(GROUPS)
    if tiny_io:
        nc.declare_dram_parameter("tiny", [128, 4], DT, isOutput=False)
        lhsT = nc.dram_tensor("lhsT", [128, 2, 1024], F8)
        rhs = nc.dram_tensor("rhs", [128, 2, N_TOTAL], F8)
        ubc = nc.dram_tensor("ubc", [128, N_TOTAL], BF)
        sv = nc.dram_tensor("sv", [128, TIERS, 4], DT)
        d_o = [nc.dram_tensor(f"d{g}", [128, GW], BF) for g in range(NG)]
        p_o = [nc.dram_tensor(f"p{g}", [128, GW], BF) for g in range(NG)]
        done_o = nc.declare_dram_parameter("done_o", [128, 4], DT,
                                           isOutput=True)
    else:
        lhsT = nc.declare_dram_parameter("lhsT", [128, 2, 1024], F8,
                                         isOutput=False)
        rhs = nc.declare_dram_parameter("rhs", [128, 2, N_TOTAL], F8,
                                        isOutput=False)
        ubc = nc.declare_dram_parameter("ubc", [128, N_TOTAL], BF,
                                        isOutput=False)
        sv = nc.declare_dram_parameter("sv", [128, TIERS, 4], DT,
                                       isOutput=False)
        d_o = [nc.declare_dram_parameter(f"d{g}", [128, GW], BF,
                                         isOutput=True) for g in range(NG)]
        p_o = [nc.declare_dram_parameter(f"p{g}", [128, GW], BF,
                                         isOutput=True) for g in range(NG)]
        done_o = None

    NIN = 4 * 16
    lay, tiles = _group_layout()
    NTILE = len(tiles)             # 9
    gstart = [0, 3, 6]
    gend = [3, 6, 9]

    # matmul segments per tile: [(psum_off, abs_col)] in 512 steps; a tile
    # may span several tiers but matmul only needs the right lhsT block.
    def tile_segs(gi, xoff):
        segs = []
        for s in range(0, 4096, 512):
            t, acol = _col_of(lay, gi, xoff + s)
            segs.append((s, t, acol))
        return segs

    from contextlib import ExitStack
    with ExitStack() as ctx:
        block = ctx.enter_context(nc.Block())
        dma_in = ctx.enter_context(nc.semaphore("dma_in"))
        pe_s = ctx.enter_context(nc.semaphore("pe_s"))
        x_s = ctx.enter_context(nc.semaphore("x_s"))
        ey_s = ctx.enter_context(nc.semaphore("ey_s"))
        ab_s = ctx.enter_context(nc.semaphore("ab_s"))
        h_s = ctx.enter_context(nc.semaphore("h_s"))
        lm_s = ctx.enter_context(nc.semaphore("lm_s"))
        d_s = ctx.enter_context(nc.semaphore("d_s"))
        dma_o = ctx.enter_context(nc.semaphore("dma_o"))
        t_l = ctx.enter_context(nc.sbuf_tensor("t_l", [128, 2, 1024], F8))
        t_r = ctx.enter_context(nc.sbuf_tensor("t_r", [128, 2, N_TOTAL], F8))
        t_u = ctx.enter_context(nc.sbuf_tensor("t_u", [128, N_TOTAL], BF))
        t_sv = ctx.enter_context(nc.sbuf_tensor("t_sv", [128, TIERS, 4], DT))
        X = ctx.enter_context(nc.sbuf_tensor("X", [128, GW], DT))
        Y = ctx.enter_context(nc.sbuf_tensor("Y", [128, GW], BF))
        LB0 = ctx.enter_context(nc.sbuf_tensor("LB0", [128, GW], BF))
        LB1 = ctx.enter_context(nc.sbuf_tensor("LB1", [128, GW], BF))
        PB0 = ctx.enter_context(nc.sbuf_tensor("PB0", [128, GW], BF))
        PB1 = ctx.enter_context(nc.sbuf_tensor("PB1", [128, GW], BF))
        ps = ctx.enter_context(nc.psum_tensor("ps", [128, 4096], DT))

        LB = [LB0, LB1]
        PB = [PB0, PB1]

        @block.sync
        def _(sync):
            for t, src in [(t_l, lhsT), (t_r, rhs), (t_u, ubc), (t_sv, sv)]:
                sync.dma_start(out=t[:], in_=src[:]).then_inc(dma_in, 16)
            for r in range(reps):
                for gi in range(NG):
                    h = r * NG + gi
                    sync.wait_ge(d_s, 2 * h + 1)
                    sync.dma_start(out=d_o[gi][:],
                                   in_=LB[h % 2][:]).then_inc(dma_o, 16)
                    sync.wait_ge(d_s, 2 * h + 2)
                    sync.dma_start(out=p_o[gi][:],
                                   in_=PB[h % 2][:]).then_inc(dma_o, 16)
            sync.wait_ge(dma_o, 32 * NG * reps + (16 if bench else 0))

        @block.tensor
        def _(te):
            te.wait_ge(dma_in, NIN)
            for r in range(reps):
                for k, (gi, xoff) in enumerate(tiles):
                    g = r * NTILE + k
                    if g >= 1:
                        te.wait_ge(x_s, g)
                    for (s, t, acol) in tile_segs(gi, xoff):
                        mm = te.matmul(
                            ps[:, s:s + 512],
                            t_l[:, :, t * 128:(t + 1) * 128],
                            t_r[:, :, acol:acol + 512],
                            start=True, stop=True, perf_mode=DR)
                    mm.then_inc(pe_s, 1)

        @block.vector
        def _(v):
            for r in range(reps):
                for gi in range(NG):
                    h = r * NG + gi
                    # sttA per psum tile: X = ubc + ps   (only psum readers)
                    for k in range(gstart[gi], gend[gi]):
                        _, xoff = tiles[k]
                        g = r * NTILE + k
                        if h >= 1 and k == gstart[gi]:
                            v.wait_ge(d_s, 2 * h)   # X/Y free (prev grp done)
                        v.wait_ge(pe_s, g + 1)
                        # ubc slice may span tiers; column-mapped per piece
                        pieces = []
                        s0 = 0
                        while s0 < 4096:
                            t, acol = _col_of(lay, gi, xoff + s0)
                            tl, tloff, tW, tc0 = next(
                                p for p in lay[gi] if p[0] == t)
                            avail = min(4096 - s0, tloff + tW - (xoff + s0))
                            pieces.append((s0, acol, avail))
                            s0 += avail
                        if len(pieces) == 1:
                            s0, acol, w = pieces[0]
                            v.scalar_tensor_tensor(
                                out=X[:, xoff:xoff + 4096],
                                in0=t_u[:, acol:acol + 4096],
                                scalar=0.0,
                                in1=ps[:], op0=OP.add,
                                op1=OP.add).then_inc(x_s, 1)
                        else:
                            for i, (s0, acol, w) in enumerate(pieces):
                                mmop = v.scalar_tensor_tensor(
                                    out=X[:, xoff + s0:xoff + s0 + w],
                                    in0=t_u[:, acol:acol + w],
                                    scalar=0.0,
                                    in1=ps[:, s0:s0 + w], op0=OP.add,
                                    op1=OP.add)
                            mmop.then_inc(x_s, 1)
                    # sttB per tier: Y = (ubc * -(1-s)) + X
                    for j, (t, loff, W, c0) in enumerate(lay[gi]):
                        v.scalar_tensor_tensor(
                            out=Y[:, loff:loff + W],
                            in0=t_u[:, c0:c0 + W],
                            scalar=t_sv[:, t, 1:2],
                            in1=X[:, loff:loff + W],
                            op0=OP.mult, op1=OP.add).then_inc(ey_s, 1)
                    # H = a - b
                    v.wait_ge(ab_s, h + 1)
                    v.tensor_sub(out=X[:], in0=X[:],
                                 in1=Y[:]).then_inc(h_s, 1)
                    # dists = LP - LM ; probs = PB*-0.5 + 0.5
                    v.wait_ge(lm_s, h + 1)
                    if h >= 2:
                        v.wait_ge(dma_o, 32 * (h - 2) + 16)  # LB[h%2] DMA'd
                    v.tensor_sub(out=LB[h % 2][:], in0=X[:],
                                 in1=Y[:]).then_inc(d_s, 1)
                    v.tensor_scalar(out=PB[h % 2][:], in0=PB[h % 2][:],
                                    scalar1=-0.5, scalar2=0.5,
                                    op0=OP.mult,
                                    op1=OP.add).then_inc(d_s, 1)

        @block.scalar
        def _(sc):
            for r in range(reps):
                for gi in range(NG):
                    h = r * NG + gi
                    ntier = len(lay[gi])
                    base_ey = (r * TIERS
                               + sum(len(lay[q]) for q in range(gi)))
                    # a per tier (needs that tier's sttB done reading X)
                    for j, (t, loff, W, c0) in enumerate(lay[gi]):
                        sc.wait_ge(ey_s, base_ey + j + 1)
                        sc.activation(X[:, loff:loff + W],
                                      X[:, loff:loff + W], F.Ln,
                                      bias=t_sv[:, t, 0:1], scale=1.0 / SC2)
                    # b per group
                    sc.activation(Y[:], Y[:], F.Ln, bias=1.0,
                                  scale=1.0 / SC2).then_inc(ab_s, 1)
                    sc.wait_ge(h_s, h + 1)
                    if h >= 2:
                        sc.wait_ge(dma_o, 32 * (h - 1))   # PB[h%2] DMA done
                    sc.activation(PB[h % 2][:], X[:], F.Exp,
                                  bias=0.0, scale=0.5)
                    sc.activation(X[:], PB[h % 2][:], F.Ln,
                                  bias=1.0, scale=1.0)
                    sc.activation(Y[:], PB[h % 2][:], F.Ln,
                                  bias=1.0, scale=-1.0).then_inc(lm_s, 1)

        @block.gpsimd
        def _(gp):
            if bench:
                gp.wait_ge(d_s, 2 * NG * reps)
                gp.memset(X[:, 0:4], 0.0)
                gp.dma_start(out=done_o[:],
                             in_=X[:, 0:4]).then_inc(dma_o, 16)

    return nc


def _prepare_in_maps(embeddings):
    import ml_dtypes
    f8 = ml_dtypes.float8_e4m3
    bf = ml_dtypes.bfloat16

    E = np.ascontiguousarray(embeddings, dtype=np.float32)
    u = (E.astype(np.float64) ** 2).sum(axis=1)          # [n]

    ET = E.T.astype(np.float64)                          # [256, n]
    rhs = np.ascontiguousarray(
        (SC * ET).astype(np.float32).astype(f8)
        .reshape(2, 128, N_TOTAL).transpose(1, 0, 2))
    lhs_all = (-2.0 * SC * ET).astype(np.float32).astype(f8)   # [256, n]

    ubc = np.ascontiguousarray(np.broadcast_to(
        (SC2 * u).astype(np.float32).astype(bf)[None, :], (128, N_TOTAL)))

    in_maps = []
    for c in range(8):
        gidx = np.concatenate(
            [np.arange((8 * t + c) * 128, (8 * t + c) * 128 + 128)
             for t in range(TIERS)])
        lhsT = np.ascontiguousarray(
            lhs_all[:, gidx].reshape(2, 128, 1024).transpose(1, 0, 2))
        sv = np.zeros((128, TIERS, 4), np.float32)
        for t in range(TIERS):
            sblk = u[(8 * t + c) * 128:(8 * t + c) * 128 + 128]
            sv[:, t, 0] = sblk                # a bias: ln(X*2^-10 + s)
            sv[:, t, 1] = -(1.0 - sblk)       # sttB: ubc * -(1-s)
        in_maps.append({"lhsT": lhsT, "rhs": rhs, "ubc": ubc,
                        "sv": np.ascontiguousarray(sv)})
    return in_maps


def kernel(embeddings: np.ndarray) -> tuple[np.ndarray, np.ndarray]:
    global _compiled
    from concourse.bass_utils import run_bass_kernel_spmd

    if _compiled is None:
        _compiled = _build_raw()
    nc = _compiled

    in_maps = _prepare_in_maps(embeddings)
    res = run_bass_kernel_spmd(nc, in_maps, list(range(8)))

    lay, _ = _group_layout()
    dists = np.zeros((N_TOTAL, N_TOTAL), np.float32)
    probs = np.zeros((N_TOTAL, N_TOTAL), np.float32)
    for c in range(8):
        for gi, lg in enumerate(lay):
            dg = np.asarray(res.results[c][f"d{gi}"]).astype(np.float32)
            pg = np.asarray(res.results[c][f"p{gi}"]).astype(np.float32)
            for (t, loff, W, c0) in lg:
                r0 = (8 * t + c) * 128
                dists[r0:r0 + 128, c0:c0 + W] = dg[:, loff:loff + W]
                probs[r0:r0 + 128, c0:c0 + W] = pg[:, loff:loff + W]

    du = np.triu(dists, 1)
    dists = du + du.T
    pu = np.triu(probs, 1)
    probs = pu + pu.T
    return (probs, dists)
